# revision 18
# baseline (speedup 1.0000x reference)
"""Trainium2 Bass kernel: Mistral-style GQA attention with sliding-window mask.

Problem: hidden [1,2048,4096] -> Wq/Wk/Wv projections (32 q heads, 8 kv heads,
head_dim 128) -> RoPE -> sliding-window (1024) causal attention -> Wo.

Sharding: tensor-parallel over heads on 8 NeuronCores. Core i owns KV head i
and query heads 4i..4i+3 (Wq/Wk/Wv row-sharded, Wo column-sharded). Each core
computes partial_i = attn_heads_i @ Wo_i^T in HBM; host sums the 8 partials
(the TP all-reduce) to produce the full output.

All inputs are pre-cast to bf16 and pre-transposed on the host; each core
writes a bf16 [2048, 4096] partial that the host accumulates in fp32.

On-device per core the work is a single software-pipelined stream per
512-query chunk, woven so PE never waits on a phase boundary:
  F(c-1): KV^T projection sweep for chunk c (2 PSUM banks), woven into the
          previous chunk's second attention head-pair as PE filler.
  B..E:   one Q^T projection sweep per query head (ring of 2 PSUM banks);
          RoPE of head h runs on DVE underneath head h+1's sweep; the
          previous chunk's 32 Wo output-tile groups (4 matmuls each, 2
          persistent PSUM banks, ACT stages them out) weave into sweeps
          h0/h1; attention for heads 0/1 (scores -> exp on ACT -> edge
          masks on DVE -> P@V) weaves into sweeps h2/h3.
  F(c):   attention heads 2/3 woven with chunk c+1's KV sweep.
Softmax denominators: exp tiles are summed into a bf16 accumulator on DVE
(all but the last two key-blocks, which ride the PE directly), then folded
into one persistent PSUM bank with a single ones-matmul per head;
reciprocal on DVE, partition-broadcast on GpSimd, normalize on DVE.
"""

import sys

for _p in ("/opt/trn_rl_repo", "/root/.axon_site/_ro/trn_rl_repo"):
    if _p not in sys.path:
        sys.path.insert(0, _p)

import numpy as np
import ml_dtypes

import concourse.bass as bass  # noqa: F401  (registers engine classes)
import concourse.mybir as mybir
import concourse.tile as tile
from concourse import bacc
from concourse.bass_utils import run_bass_kernel_spmd

S = 2048
HID = 4096
D = 128
NQH = 4          # query heads per core
NCORES = 8
SC = 512         # seq chunk
NCH = S // SC
KT = HID // 128  # contraction tiles
WINDOW = 1024
ROPE_BASE = 10000.0
SCALE = 1.0 / float(np.sqrt(D))

F32 = mybir.dt.float32
BF16 = mybir.dt.bfloat16
FP8 = mybir.dt.float8e4
DR = mybir.MatmulPerfMode.DoubleRow
MULT = mybir.AluOpType.mult
ADD = mybir.AluOpType.add
SUB = mybir.AluOpType.subtract
EXP = mybir.ActivationFunctionType.Exp
IDN = mybir.ActivationFunctionType.Identity
COPY = mybir.ActivationFunctionType.Copy

# fp8 hi/lo power-of-2 scales: activations (hidden, attn-out) x32,
# weights x2048; the 2^-16 descale folds into the rope tables / the
# PSUM->SBUF staging copies.
SX = 2.0 ** 5
SWT = 2.0 ** 11
SA = 2.0 ** 5
DSC = 1.0 / (SX * SWT)     # proj descale (V path; Q/K fold into rope tables)
WSC = 1.0 / (SA * SWT)     # Wo descale

# slot sl = kb - 4c + 8 for key-block kb in chunk c; exp-written region.
def _slot_region(sl):
    lo = 128 * (sl - 8) if sl >= 8 else 0
    hi = 512 if sl >= 3 else 128 * (sl + 1)
    return lo, hi


def _chunk_kbs(c):
    return list(range(max(0, 4 * c - 8), 4 * c + 4))


def _program(tc, dr, out, niter=1, fused=True):
    nc = tc.nc
    hth, htl = dr["hth"], dr["htl"]
    d_wqh, d_wql = dr["wqh"], dr["wql"]
    d_wkh, d_wkl = dr["wkh"], dr["wkl"]
    d_wvh, d_wvl = dr["wvh"], dr["wvl"]
    d_woh, d_wol = dr["woh"], dr["wol"]
    ctab, stab, mcaus, mwin = dr["ctab"], dr["stab"], dr["mcaus"], dr["mwin"]

    from contextlib import ExitStack
    with ExitStack() as ctx:
        pw = ctx.enter_context(tc.tile_pool(name="persist", bufs=1))
        prt = ctx.enter_context(tc.tile_pool(name="ropet", bufs=1))

        # fp8 hi/lo weight planes, host-laid-out so DoubleRow slab-pair
        # APs are contiguous 2x128 / 2x512 blocks
        wqh = pw.tile([128, KT * 512], FP8, name="wqh")
        wql = pw.tile([128, KT * 512], FP8, name="wql")
        wkh = pw.tile([128, KT * 128], FP8, name="wkh")
        wkl = pw.tile([128, KT * 128], FP8, name="wkl")
        wvh = pw.tile([128, KT * 128], FP8, name="wvh")
        wvl = pw.tile([128, KT * 128], FP8, name="wvl")
        woh = pw.tile([128, NQH * HID], FP8, name="woh")
        wol = pw.tile([128, NQH * HID], FP8, name="wol")
        qtb = [pw.tile([128, S], BF16, name=f"qtb{h}") for h in range(NQH)]
        ktb = pw.tile([128, S], BF16, name="ktb")
        vtb = pw.tile([128, S], BF16, name="vtb")
        vnat = pw.tile([128, S], BF16, name="vnat")
        cs_t = pw.tile([128, S], F32, name="cs_t")
        sn_t = pw.tile([128, S], F32, name="sn_t")
        mc_t = pw.tile([128, 128], BF16, name="mc_t")
        mw_t = pw.tile([128, 128], BF16, name="mw_t")
        ones_t = pw.tile([128, 1], BF16, name="ones_t")

        nc.gpsimd.memset(ones_t[:], 1.0)

        def _rope(dst, p, c):
            """dst[bf16 [128,512] slice] = rope(p [128,512], PSUM), chunk c.

            cs_t is cos duplicated across both halves; sn_t is sign-baked
            sin: rows 0:64 = -sin, rows 64:128 = +sin, so
            out = q*cos + rot(q)*sn with rot a plain half-swap.
            """
            lo, hi = SC * c, SC * (c + 1)
            csl = cs_t[:, lo:hi]
            snl = sn_t[:, lo:hi]
            e = nc.vector
            w = 512
            t1 = prt.tile([64, w], F32, tag="rt1", name="rt1")
            t2 = prt.tile([64, w], F32, tag="rt2", name="rt2")
            e.tensor_tensor(t1[:], p[0:64, :], csl[0:64, :], MULT)
            e.tensor_tensor(t2[:], p[64:128, :], snl[0:64, :], MULT)
            e.tensor_tensor(dst[0:64, :], t1[:], t2[:], ADD)
            t3 = prt.tile([64, w], F32, tag="rt1", name="rt3")
            t4 = prt.tile([64, w], F32, tag="rt2", name="rt4")
            e.tensor_tensor(t3[:], p[64:128, :], csl[64:128, :], MULT)
            e.tensor_tensor(t4[:], p[0:64, :], snl[64:128, :], MULT)
            e.tensor_tensor(dst[64:128, :], t3[:], t4[:], ADD)

        phb = ctx.enter_context(tc.tile_pool(name="htbp", bufs=12))
        ppt = ctx.enter_context(tc.tile_pool(name="ptp", bufs=6))
        pmisc = ctx.enter_context(tc.tile_pool(name="miscb", bufs=2))
        pdac = ctx.enter_context(tc.tile_pool(name="daccb", bufs=2))
        pat = ctx.enter_context(tc.tile_pool(name="atbp", bufs=8))
        posb = ctx.enter_context(tc.tile_pool(name="osbp", bufs=2))
        # persistent PSUM: denominator rows (1 bank) + the Q-projection
        # ring (2 banks; persistent so a chunk's first Q matmul WAR-waits
        # only on the previous chunk's long-finished rope reads, not on the
        # attention drain). The Wo ring is a per-chunk 2-bank pool open only
        # during the h0/h1 sweeps.
        pdn = ctx.enter_context(tc.tile_pool(name="denps", bufs=1, space="PSUM"))
        pq = ctx.enter_context(tc.tile_pool(name="qps", bufs=2, space="PSUM"))
        pdnt = pdn.tile([33, 512], F32, name="pdnt")

        hbt = {}      # (c, g) -> hidden-tile [128, 4*512]
        kvt = {}      # c -> (kpp, vpp) PSUM tiles

        def issue_hb(c, g):
            # [Xh slices k0..k3 | Xl slices k0..k3], fp8
            t = phb.tile([128, 4096], FP8, tag="htb", name="hb", bufs=12)
            for pl, srch in ((0, hth), (1, htl)):
                nc.sync.dma_start(
                    t[:, 2048 * pl:2048 * (pl + 1)].rearrange(
                        "p (k j) -> p k j", j=512),
                    srch[512 * g:512 * (g + 1), SC * c:SC * (c + 1)].rearrange(
                        "(k p) j -> p k j", p=128))
            hbt[(c, g)] = t

        def _pair2(ap):
            return ap.rearrange("p (two f) -> p two f", two=2)

        def _hpair(c, p):
            # moving slab-pair APs (Xh, Xl) for global slab pair p
            t = hbt[(c, p // 2)]
            i = p % 2
            return (_pair2(t[:, 1024 * i:1024 * (i + 1)]),
                    _pair2(t[:, 2048 + 1024 * i:2048 + 1024 * (i + 1)]))

        def _mm3(out_ap, wh_ap, wl_ap, mh, ml, first, last):
            # 3-term hi/lo fp8 product over one slab pair: main + both
            # single residuals (the Wl*Xl term is ~1e-3 relative, dropped)
            nc.tensor.matmul(out_ap, wh_ap, mh, start=first, stop=False,
                             perf_mode=DR, skip_group_check=True)
            nc.tensor.matmul(out_ap, wl_ap, mh, start=False, stop=False,
                             perf_mode=DR, skip_group_check=True)
            nc.tensor.matmul(out_ap, wh_ap, ml, start=False, stop=last,
                             perf_mode=DR, skip_group_check=True)

        def kv_sweep(c, pkv):
            kpp = pkv.tile([128, 512], F32, tag="kpp", name="kpp")
            vpp = pkv.tile([128, 512], F32, tag="vpp", name="vpp")
            kvt[c] = (kpp, vpp)
            for p in range(KT // 2):
                mh, ml = _hpair(c, p)
                first, last = p == 0, p == KT // 2 - 1
                _mm3(kpp[:], _pair2(wkh[:, 256 * p:256 * (p + 1)]),
                     _pair2(wkl[:, 256 * p:256 * (p + 1)]), mh, ml, first, last)
                _mm3(vpp[:], _pair2(wvh[:, 256 * p:256 * (p + 1)]),
                     _pair2(wvl[:, 256 * p:256 * (p + 1)]), mh, ml, first, last)
                yield

        def q_sweep(c, h, qpp):
            for p in range(KT // 2):
                mh, ml = _hpair(c, p)
                off = 1024 * p + 256 * h
                _mm3(qpp[:], _pair2(wqh[:, off:off + 256]),
                     _pair2(wql[:, off:off + 256]), mh, ml,
                     p == 0, p == KT // 2 - 1)
                yield

        wo_state = {"obig": None, "pool": None, "bufs": 2}

        def wo_groups(c, atbs, lo_m, hi_m):
            # one group = one [128q, 512hid] output tile of chunk c
            for m in range(lo_m, hi_m):
                wj, wn = m // 8, m % 8
                if wn % 2 == 0:
                    wo_state["obig"] = posb.tile([128, 1024], BF16,
                                                 tag="osb", name="osb")
                obig = wo_state["obig"]
                ath_t, atl_t = atbs
                po = wo_state["pool"].tile([128, 512], F32, tag="po",
                                            name="po", bufs=wo_state["bufs"])
                for hp in range(2):
                    soff = 1024 * hp + 256 * wj
                    moff = 8192 * hp + 1024 * wn
                    sm = _pair2(ath_t[:, soff:soff + 256])
                    sl_ = _pair2(atl_t[:, soff:soff + 256])
                    mh = _pair2(woh[:, moff:moff + 1024])
                    ml = _pair2(wol[:, moff:moff + 1024])
                    _mm3(po[:], sm, sl_, mh, ml, hp == 0, hp == 1)
                nc.scalar.activation(obig[:, 512 * (wn % 2):512 * (wn % 2 + 1)],
                                     po[:], IDN, scale=WSC)
                if wn % 2 == 1:
                    nc.scalar.dma_start(
                        out[SC * c + 128 * wj:SC * c + 128 * (wj + 1),
                            1024 * (wn // 2):1024 * (wn // 2 + 1)],
                        obig[:])
                yield

        def _at_head_ap(t, h):
            hp, pl = h // 2, h % 2
            off = 1024 * hp + 128 * pl
            return t[:, off:off + 1024].rearrange(
                "p (b s) -> p b s", b=4)[:, :, 0:128]

        def attn_head(c, h, psc, ppv, atbs_out):
            kbs = _chunk_kbs(c)
            first_kb, last_kb = kbs[0], kbs[-1]
            acc_kbs = kbs[:-2]
            tail_kbs = kbs[-2:]
            dacc = pdac.tile([128, 512], BF16, tag="dac", name="dac", bufs=2)
            nc.vector.memset(dacc[:], 0.0)
            den = pdnt[32 * (h % 2):32 * (h % 2) + 1, :]
            pvt = ppv.tile([128, 512], F32, tag="pv", name="pv", bufs=1)

            def emit_pv(kb, pt):
                lo, hi = _slot_region(kb - 4 * c + 8)
                nc.tensor.matmul(pvt[:, lo:hi], vnat[:, 128 * kb:128 * (kb + 1)],
                                 pt[:, lo:hi], start=(kb == first_kb),
                                 stop=(kb == last_kb), skip_group_check=True)
                if kb in tail_kbs:
                    nc.tensor.matmul(den[:, lo:hi], ones_t[:], pt[:, lo:hi],
                                     start=False, stop=(kb == last_kb),
                                     skip_group_check=True)

            pending = []
            for kb in kbs:
                sl = kb - 4 * c + 8
                lo, hi = _slot_region(sl)
                sct = psc.tile([128, 512], F32, tag="sc", name="sc", bufs=2)
                nc.tensor.matmul(sct[:, lo:hi], ktb[:, 128 * kb:128 * (kb + 1)],
                                 qtb[h][:, SC * c + lo:SC * c + hi],
                                 start=True, stop=True)
                pt = ppt.tile([128, 512], BF16, tag="pt", name="pt", bufs=6)
                nc.scalar.activation(pt[:, lo:hi], sct[:, lo:hi], EXP, scale=SCALE)
                if sl <= 3:
                    mofs = 128 * sl
                    nc.vector.tensor_tensor(pt[:, mofs:mofs + 128],
                                            pt[:, mofs:mofs + 128], mw_t[:], MULT)
                elif sl >= 8:
                    mofs = 128 * (sl - 8)
                    nc.vector.tensor_tensor(pt[:, mofs:mofs + 128],
                                            pt[:, mofs:mofs + 128], mc_t[:], MULT)
                if kb in acc_kbs:
                    nc.vector.tensor_tensor(dacc[:, lo:hi], dacc[:, lo:hi],
                                            pt[:, lo:hi], ADD)
                pending.append((kb, pt))
                if len(pending) > 2:
                    emit_pv(*pending.pop(0))
                yield
            # fold the accumulated exp sums into the denominator row; the two
            # tail key-blocks accumulate directly in emit_pv
            nc.tensor.matmul(den, ones_t[:], dacc[:], start=True, stop=False,
                             skip_group_check=True)
            for item in pending:
                emit_pv(*item)
                yield
            # drain: stage P@V out via ACT so the bank frees, normalize off
            # the critical path
            pvu = pat.tile([128, 512], BF16, tag="pvu", name="pvu", bufs=2)
            nc.scalar.activation(pvu[:], pvt[:], IDN, scale=SA)
            dre = pmisc.tile([1, 512], BF16, tag="denr", name="denr")
            with nc.allow_low_precision(reason="softmax denom to bf16"):
                nc.vector.reciprocal(dre[:], den)
            dbc = pmisc.tile([128, 512], BF16, tag="denb", name="denb")
            nc.gpsimd.partition_broadcast(dbc[:], dre[:])
            at = pat.tile([128, 512], BF16, tag="atb", name="atb", bufs=2)
            nc.vector.tensor_tensor(at[:], pvu[:], dbc[:], MULT)
            ath_t, atl_t = atbs_out
            atv = at[:].rearrange("p (b s) -> p b s", s=128)
            hap = _at_head_ap(ath_t, h)
            nc.scalar.activation(hap, atv, COPY)
            nc.vector.tensor_tensor(_at_head_ap(atl_t, h), atv, hap, SUB)

        def chain(*gens):
            for g in gens:
                yield from g

        def drive(*gens_weights):
            gens = list(gens_weights)
            while gens:
                keep = []
                for g, wgt in gens:
                    alive = True
                    for _ in range(wgt):
                        try:
                            next(g)
                        except StopIteration:
                            alive = False
                            break
                    if alive:
                        keep.append((g, wgt))
                gens = keep

        def empty_gen():
            return iter(())

        # ---- initial DMA stream (weights + chunk-0 hidden tiles) ----
        # weight dram arrays are exact SBUF images (host pre-laid-out)
        for g in range(4):
            qs = 4096 * g
            ks = 1024 * g
            nc.sync.dma_start(wkh[:, ks:ks + 1024], d_wkh[:, ks:ks + 1024])
            nc.sync.dma_start(wkl[:, ks:ks + 1024], d_wkl[:, ks:ks + 1024])
            nc.sync.dma_start(wvh[:, ks:ks + 1024], d_wvh[:, ks:ks + 1024])
            nc.sync.dma_start(wvl[:, ks:ks + 1024], d_wvl[:, ks:ks + 1024])
            issue_hb(0, g)
            nc.sync.dma_start(wqh[:, qs:qs + 4096], d_wqh[:, qs:qs + 4096])
            nc.sync.dma_start(wql[:, qs:qs + 4096], d_wql[:, qs:qs + 4096])
        nc.sync.dma_start(cs_t[:], ctab[:])
        nc.sync.dma_start(sn_t[:], stab[:])
        nc.sync.dma_start(mc_t[:], mcaus[:])
        nc.sync.dma_start(mw_t[:], mwin[:])
        for g in range(4, 8):
            issue_hb(0, g)
        for j in range(8):
            js = 2048 * j
            nc.sync.dma_start(woh[:, js:js + 2048], d_woh[:, js:js + 2048])
            nc.sync.dma_start(wol[:, js:js + 2048], d_wol[:, js:js + 2048])

        # ---- bootstrap: KV sweep of chunk 0 ----
        pkv_pools = {}
        pkv_pools[0] = tc.alloc_tile_pool(name="kvps0", bufs=1, space="PSUM")
        drive((kv_sweep(0, pkv_pools[0]), 1))

        atbs_prev = None     # previous chunk's normalized attention tiles
        prev_c = None
        seq = [(it, c) for it in range(niter) for c in range(NCH)]
        for idx, (it, c) in enumerate(seq):
            nxt = seq[idx + 1] if idx + 1 < len(seq) else None
            kpp, vpp = kvt.pop(c)

            # V^T -> SBUF, DMA-transpose to natural; rope K. Both read the KV
            # PSUM banks, which then free for this chunk's attention pools.
            nc.scalar.activation(vtb[:, SC * c:SC * (c + 1)], vpp[:], IDN,
                                 scale=DSC)
            for b2 in range(4):
                bo = 128 * (4 * c + b2)
                nc.scalar.dma_start_transpose(
                    vnat[:, bo:bo + 128],
                    vtb[:, SC * c + 128 * b2:SC * c + 128 * (b2 + 1)])
            _rope(ktb[:, SC * c:SC * (c + 1)], kpp[:], c)
            pkv_pools.pop(c).release()

            pwo = tc.alloc_tile_pool(name=f"wops{idx}", bufs=2, space="PSUM")
            wo_state["pool"], wo_state["bufs"] = pwo, 2
            wo_iter = (wo_groups(prev_c, atbs_prev, 0, 32) if atbs_prev is not None
                       else empty_gen())
            # fp8 hi/lo attention-output planes, head-pair interleaved at
            # 128-col granularity (padded so the strided write AP fits)
            atbs_new = (pat.tile([128, 2304], FP8, tag="ath", name="ath", bufs=2),
                        pat.tile([128, 2304], FP8, tag="atl", name="atl", bufs=2))
            psc = ppv = None
            a01 = None
            for h in range(NQH):
                if nxt is not None:
                    issue_hb(nxt[1], 2 * h)
                    issue_hb(nxt[1], 2 * h + 1)
                qpp = pq.tile([128, 512], F32, tag="qp", name="qp", bufs=2)
                if h == 2:
                    # Wo groups are done after the h0/h1 sweeps: swap their
                    # 2 banks for a deeper score ring + double P@V buffers
                    pwo.release()
                    psc = tc.alloc_tile_pool(name=f"scps{idx}", bufs=2, space="PSUM", side="right")
                    ppv = tc.alloc_tile_pool(name=f"pvps{idx}", bufs=1, space="PSUM", side="right")
                    a01 = chain(attn_head(c, 0, psc, ppv, atbs_new),
                                attn_head(c, 1, psc, ppv, atbs_new))
                fill = wo_iter if h < 2 else a01
                drive((q_sweep(c, h, qpp), 2), (fill, 1))
                _rope(qtb[h][:, SC * c:SC * (c + 1)], qpp[:], c)
            # F: finish heads 0/1, run heads 2/3, weave next chunk's KV sweep
            a23 = chain(attn_head(c, 2, psc, ppv, atbs_new),
                        attn_head(c, 3, psc, ppv, atbs_new))
            if nxt is not None:
                pkv_pools[nxt[1]] = tc.alloc_tile_pool(
                    name=f"kvps{idx + 1}", bufs=1, space="PSUM")
                kv_fill = kv_sweep(nxt[1], pkv_pools[nxt[1]])
            else:
                kv_fill = empty_gen()
            drive((a01, 1), (wo_iter, 1), (a23, 2), (kv_fill, 1))
            ppv.release()
            psc.release()

            atbs_prev = atbs_new
            prev_c = c

        # drain the last chunk's Wo groups on a wider PSUM ring
        pdr = tc.alloc_tile_pool(name="drainps", bufs=4, space="PSUM")
        wo_state["pool"], wo_state["bufs"] = pdr, 4
        drive((wo_groups(prev_c, atbs_prev, 0, 32), 1))
        pdr.release()


_NC_CACHE = {}


def _build(niter=1, fused=True):
    import os
    fused = os.environ.get("KERNEL_FUSED", "1" if fused else "0") == "1"
    key = (niter, fused)
    if key in _NC_CACHE:
        return _NC_CACHE[key]
    nc = bacc.Bacc("TRN2", target_bir_lowering=False, debug=False,
                   enable_asserts=True, num_devices=NCORES)
    dr = {}

    def din(name, shape, dt=F32):
        dr[name] = nc.dram_tensor(name, shape, dt, kind="ExternalInput").ap()

    din("hth", [HID, S], FP8)
    din("htl", [HID, S], FP8)
    din("wqh", [128, KT * 512], FP8)
    din("wql", [128, KT * 512], FP8)
    din("wkh", [128, KT * 128], FP8)
    din("wkl", [128, KT * 128], FP8)
    din("wvh", [128, KT * 128], FP8)
    din("wvl", [128, KT * 128], FP8)
    din("woh", [128, NQH * HID], FP8)
    din("wol", [128, NQH * HID], FP8)
    din("ctab", [128, S])
    din("stab", [128, S])
    din("mcaus", [128, 128], BF16)
    din("mwin", [128, 128], BF16)
    out = nc.dram_tensor("out", [S, HID], BF16, kind="ExternalOutput").ap()

    with tile.TileContext(nc) as tc:
        _program(tc, dr, out, niter, fused)
    nc.compile()
    _NC_CACHE[key] = nc
    return nc


def make_in_maps(inputs):
    hs = np.asarray(inputs["hidden_states"], dtype=np.float32)
    Wq = np.asarray(inputs["Wq"], dtype=np.float32)
    Wk = np.asarray(inputs["Wk"], dtype=np.float32)
    Wv = np.asarray(inputs["Wv"], dtype=np.float32)
    Wo = np.asarray(inputs["Wo"], dtype=np.float32)
    pos = np.asarray(inputs["position_ids"]).reshape(-1)

    assert hs.shape == (1, S, HID), hs.shape
    H = hs[0]
    HT = np.ascontiguousarray(H.T)

    E4 = ml_dtypes.float8_e4m3

    def hilo(x):
        xh = x.astype(E4)
        xl = (x - xh.astype(np.float32)).astype(E4)
        return xh, xl

    # hidden-state hi/lo fp8 planes, scaled by SX
    hth, htl = hilo(HT * np.float32(SX))

    # RoPE tables in [d%64, s] layout; the 1/(SX*SWT) projection descale is
    # folded in (cos duplicated, sin sign-baked)
    inv = (1.0 / (ROPE_BASE ** (np.arange(0, D, 2, dtype=np.float32) / D))).astype(np.float32)
    ang = pos.astype(np.float32)[None, :] * inv[:, None]          # [64, S]
    dsc = np.float32(1.0 / (SX * SWT))
    cos64 = (np.cos(ang) * dsc).astype(np.float32)
    sin64 = (np.sin(ang) * dsc).astype(np.float32)
    ctab = np.concatenate([cos64, cos64], axis=0)                 # [128, S]
    stab = np.concatenate([-sin64, sin64], axis=0)                # sign-baked

    kk = np.arange(128)[:, None]
    qq = np.arange(128)[None, :]
    mcaus = (qq >= kk).astype(ml_dtypes.bfloat16)   # causal diag block, [k,q]
    mwin = (qq < kk).astype(ml_dtypes.bfloat16)     # window-edge block, [k,q]

    def q_image(Wshard):
        # SBUF image [128, KT*512]: col(pair, head, plane, j) =
        # 1024*pair + 256*head + 128*plane + j; partition = hid%128
        Ws = (Wshard.T * np.float32(SWT)).astype(np.float32)      # [HID, 512]
        X = Ws.reshape(KT // 2, 2, 128, NQH, 128)                 # [pr, pl, p, h, j]
        return np.ascontiguousarray(
            X.transpose(2, 0, 3, 1, 4).reshape(128, KT * 512))

    def kv_image(Wshard):
        # [128, KT*128]: col(pair, plane, j) = 256*pair + 128*plane + j
        Ws = (Wshard.T * np.float32(SWT)).astype(np.float32)      # [HID, 128]
        X = Ws.reshape(KT // 2, 2, 128, 128)                      # [pr, pl, p, j]
        return np.ascontiguousarray(
            X.transpose(2, 0, 1, 3).reshape(128, KT * 128))

    def o_image(Wshard_cols):
        # moving image [128, NQH*HID]: col(hp, wn, plane, j) =
        # 8192*hp + 1024*wn + 512*plane + j; partition = d within head
        Ws = (Wshard_cols.T * np.float32(SWT)).astype(np.float32)  # [512, HID]
        X = Ws.reshape(2, 2, 128, 8, 512)                          # [hp, pl, p, wn, j]
        return np.ascontiguousarray(
            X.transpose(2, 0, 3, 1, 4).reshape(128, NQH * HID))

    in_maps = []
    for i in range(NCORES):
        wqh, wql = hilo(q_image(Wq[512 * i:512 * (i + 1), :]))
        wkh, wkl = hilo(kv_image(Wk[128 * i:128 * (i + 1), :]))
        wvh, wvl = hilo(kv_image(Wv[128 * i:128 * (i + 1), :]))
        woh, wol = hilo(o_image(Wo[:, 512 * i:512 * (i + 1)]))
        in_maps.append({
            "hth": hth, "htl": htl,
            "wqh": wqh, "wql": wql,
            "wkh": wkh, "wkl": wkl,
            "wvh": wvh, "wvl": wvl,
            "woh": woh, "wol": wol,
            "ctab": ctab,
            "stab": stab,
            "mcaus": mcaus,
            "mwin": mwin,
        })

    return in_maps


def kernel(**inputs):
    in_maps = make_in_maps(inputs)
    nc = _build()
    res = run_bass_kernel_spmd(nc, in_maps, core_ids=list(range(NCORES)))

    acc = np.zeros((S, HID), dtype=np.float32)
    for r in res.results:
        acc += r["out"].astype(np.float32)
    return acc.reshape(1, S, HID)


# revision 23
# speedup vs baseline: 1.0209x; 1.0209x over previous
"""Trainium2 Bass kernel: Mistral-style GQA attention with sliding-window mask.

Problem: hidden [1,2048,4096] -> Wq/Wk/Wv projections (32 q heads, 8 kv heads,
head_dim 128) -> RoPE -> sliding-window (1024) causal attention -> Wo.

Sharding: tensor-parallel over heads on 8 NeuronCores. Core i owns KV head i
and query heads 4i..4i+3 (Wq/Wk/Wv row-sharded, Wo column-sharded). Each core
computes partial_i = attn_heads_i @ Wo_i^T in HBM; host sums the 8 partials
(the TP all-reduce) to produce the full output.

All inputs are pre-cast to bf16 and pre-transposed on the host; each core
writes a bf16 [2048, 4096] partial that the host accumulates in fp32.

On-device per core the work is a single software-pipelined stream per
512-query chunk, woven so PE never waits on a phase boundary:
  F(c-1): KV^T projection sweep for chunk c (2 PSUM banks), woven into the
          previous chunk's second attention head-pair as PE filler.
  B..E:   one Q^T projection sweep per query head (ring of 2 PSUM banks);
          RoPE of head h runs on DVE underneath head h+1's sweep; the
          previous chunk's 32 Wo output-tile groups (4 matmuls each, 2
          persistent PSUM banks, ACT stages them out) weave into sweeps
          h0/h1; attention for heads 0/1 (scores -> exp on ACT -> edge
          masks on DVE -> P@V) weaves into sweeps h2/h3.
  F(c):   attention heads 2/3 woven with chunk c+1's KV sweep.
Softmax denominators: exp tiles are summed into a bf16 accumulator on DVE
(all but the last two key-blocks, which ride the PE directly), then folded
into one persistent PSUM bank with a single ones-matmul per head;
reciprocal on DVE, partition-broadcast on GpSimd, normalize on DVE.
"""

import sys

for _p in ("/opt/trn_rl_repo", "/root/.axon_site/_ro/trn_rl_repo"):
    if _p not in sys.path:
        sys.path.insert(0, _p)

import numpy as np
import ml_dtypes

import concourse.bass as bass  # noqa: F401  (registers engine classes)
import concourse.mybir as mybir
import concourse.tile as tile
from concourse import bacc
from concourse.bass_utils import run_bass_kernel_spmd

S = 2048
HID = 4096
D = 128
NQH = 4          # query heads per core
NCORES = 8
SC = 512         # seq chunk
NCH = S // SC
KT = HID // 128  # contraction tiles
WINDOW = 1024
ROPE_BASE = 10000.0
SCALE = 1.0 / float(np.sqrt(D))

F32 = mybir.dt.float32
BF16 = mybir.dt.bfloat16
FP8 = mybir.dt.float8e4
DR = mybir.MatmulPerfMode.DoubleRow
MULT = mybir.AluOpType.mult
ADD = mybir.AluOpType.add
SUB = mybir.AluOpType.subtract
EXP = mybir.ActivationFunctionType.Exp
IDN = mybir.ActivationFunctionType.Identity
COPY = mybir.ActivationFunctionType.Copy

# fp8 hi/lo power-of-2 scales: activations (hidden, attn-out) x32,
# weights x2048; the 2^-16 descale folds into the rope tables / the
# PSUM->SBUF staging copies.
SX = 2.0 ** 5
SWT = 2.0 ** 11
SA = 2.0 ** 5
DSC = 1.0 / (SX * SWT)     # proj descale (V path; Q/K fold into rope tables)
WSC = 1.0 / (SA * SWT)     # Wo descale

# slot sl = kb - 4c + 8 for key-block kb in chunk c; exp-written region.
def _slot_region(sl):
    lo = 128 * (sl - 8) if sl >= 8 else 0
    hi = 512 if sl >= 3 else 128 * (sl + 1)
    return lo, hi


def _chunk_kbs(c):
    return list(range(max(0, 4 * c - 8), 4 * c + 4))


def _program(tc, dr, out, niter=1, fused=True):
    nc = tc.nc
    hth, htl = dr["hth"], dr["htl"]
    d_wqh, d_wql = dr["wqh"], dr["wql"]
    d_wkh, d_wkl = dr["wkh"], dr["wkl"]
    d_wvh, d_wvl = dr["wvh"], dr["wvl"]
    d_woh, d_wol = dr["woh"], dr["wol"]
    ctab, stab, mcaus, mwin = dr["ctab"], dr["stab"], dr["mcaus"], dr["mwin"]

    from contextlib import ExitStack
    with ExitStack() as ctx:
        pw = ctx.enter_context(tc.tile_pool(name="persist", bufs=1))
        prt = ctx.enter_context(tc.tile_pool(name="ropet", bufs=1))

        # fp8 hi/lo weight planes, host-laid-out so DoubleRow slab-pair
        # APs are contiguous 2x128 / 2x512 blocks
        wqh = pw.tile([128, KT * 512], FP8, name="wqh")
        wql = pw.tile([128, KT * 512], FP8, name="wql")
        wkh = pw.tile([128, KT * 128], FP8, name="wkh")
        wkl = pw.tile([128, KT * 128], FP8, name="wkl")
        wvh = pw.tile([128, KT * 128], FP8, name="wvh")
        wvl = pw.tile([128, KT * 128], FP8, name="wvl")
        woh = pw.tile([128, NQH * HID], FP8, name="woh")
        wol = pw.tile([128, NQH * HID], FP8, name="wol")
        qtb = [pw.tile([128, S], BF16, name=f"qtb{h}") for h in range(NQH)]
        ktb = pw.tile([128, S], BF16, name="ktb")
        vtb = pw.tile([128, S], BF16, name="vtb")
        vnat = pw.tile([128, S], BF16, name="vnat")
        cs_t = pw.tile([128, S], F32, name="cs_t")
        sn_t = pw.tile([128, S], F32, name="sn_t")
        mc_t = pw.tile([128, 128], BF16, name="mc_t")
        mw_t = pw.tile([128, 128], BF16, name="mw_t")
        ones_t = pw.tile([128, 1], BF16, name="ones_t")

        nc.gpsimd.memset(ones_t[:], 1.0)

        def _rope(dst, p, c):
            """dst[bf16 [128,512] slice] = rope(p [128,512], PSUM), chunk c.

            cs_t is cos duplicated across both halves; sn_t is sign-baked
            sin: rows 0:64 = -sin, rows 64:128 = +sin, so
            out = q*cos + rot(q)*sn with rot a plain half-swap.
            """
            lo, hi = SC * c, SC * (c + 1)
            csl = cs_t[:, lo:hi]
            snl = sn_t[:, lo:hi]
            e = nc.vector
            w = 512
            t1 = prt.tile([64, w], F32, tag="rt1", name="rt1")
            t2 = prt.tile([64, w], F32, tag="rt2", name="rt2")
            e.tensor_tensor(t1[:], p[0:64, :], csl[0:64, :], MULT)
            e.tensor_tensor(t2[:], p[64:128, :], snl[0:64, :], MULT)
            e.tensor_tensor(dst[0:64, :], t1[:], t2[:], ADD)
            t3 = prt.tile([64, w], F32, tag="rt1", name="rt3")
            t4 = prt.tile([64, w], F32, tag="rt2", name="rt4")
            e.tensor_tensor(t3[:], p[64:128, :], csl[64:128, :], MULT)
            e.tensor_tensor(t4[:], p[0:64, :], snl[64:128, :], MULT)
            e.tensor_tensor(dst[64:128, :], t3[:], t4[:], ADD)

        phb = ctx.enter_context(tc.tile_pool(name="htbp", bufs=12))
        ppt = ctx.enter_context(tc.tile_pool(name="ptp", bufs=6))
        pmisc = ctx.enter_context(tc.tile_pool(name="miscb", bufs=2))
        pdac = ctx.enter_context(tc.tile_pool(name="daccb", bufs=2))
        pat = ctx.enter_context(tc.tile_pool(name="atbp", bufs=8))
        posb = ctx.enter_context(tc.tile_pool(name="osbp", bufs=2))
        # persistent PSUM: denominator rows (1 bank) + the Q-projection
        # ring (2 banks; persistent so a chunk's first Q matmul WAR-waits
        # only on the previous chunk's long-finished rope reads, not on the
        # attention drain). The Wo ring is a per-chunk 2-bank pool open only
        # during the h0/h1 sweeps.
        pdn = ctx.enter_context(tc.tile_pool(name="denps", bufs=1, space="PSUM"))
        pq = ctx.enter_context(tc.tile_pool(name="qps", bufs=2, space="PSUM"))
        pdnt = pdn.tile([33, 512], F32, name="pdnt")

        hbt = {}      # (c, g) -> hidden-tile [128, 4*512]
        kvt = {}      # c -> (kpp, vpp) PSUM tiles

        def issue_hb(c, g):
            # [Xh slices k0..k3 | Xl slices k0..k3], fp8
            t = phb.tile([128, 4096], FP8, tag="htb", name="hb", bufs=12)
            for pl, srch in ((0, hth), (1, htl)):
                nc.sync.dma_start(
                    t[:, 2048 * pl:2048 * (pl + 1)].rearrange(
                        "p (k j) -> p k j", j=512),
                    srch[512 * g:512 * (g + 1), SC * c:SC * (c + 1)].rearrange(
                        "(k p) j -> p k j", p=128))
            hbt[(c, g)] = t

        def _pair2(ap):
            return ap.rearrange("p (two f) -> p two f", two=2)

        def _hpair(c, p):
            # moving slab-pair APs (Xh, Xl) for global slab pair p
            t = hbt[(c, p // 2)]
            i = p % 2
            return (_pair2(t[:, 1024 * i:1024 * (i + 1)]),
                    _pair2(t[:, 2048 + 1024 * i:2048 + 1024 * (i + 1)]))

        def _mm3(out_ap, wh_ap, wl_ap, mh, ml, first, last):
            # 3-term hi/lo fp8 product over one slab pair: main + both
            # single residuals (the Wl*Xl term is ~1e-3 relative, dropped)
            nc.tensor.matmul(out_ap, wh_ap, mh, start=first, stop=False,
                             perf_mode=DR, skip_group_check=True)
            nc.tensor.matmul(out_ap, wl_ap, mh, start=False, stop=False,
                             perf_mode=DR, skip_group_check=True)
            nc.tensor.matmul(out_ap, wh_ap, ml, start=False, stop=last,
                             perf_mode=DR, skip_group_check=True)

        def kv_sweep(c, pkv):
            kpp = pkv.tile([128, 512], F32, tag="kpp", name="kpp")
            vpp = pkv.tile([128, 512], F32, tag="vpp", name="vpp")
            kvt[c] = (kpp, vpp)
            for p in range(KT // 2):
                mh, ml = _hpair(c, p)
                first, last = p == 0, p == KT // 2 - 1
                _mm3(kpp[:], _pair2(wkh[:, 256 * p:256 * (p + 1)]),
                     _pair2(wkl[:, 256 * p:256 * (p + 1)]), mh, ml, first, last)
                _mm3(vpp[:], _pair2(wvh[:, 256 * p:256 * (p + 1)]),
                     _pair2(wvl[:, 256 * p:256 * (p + 1)]), mh, ml, first, last)
                yield

        def q_sweep(c, h, qpp):
            for p in range(KT // 2):
                mh, ml = _hpair(c, p)
                off = 1024 * p + 256 * h
                _mm3(qpp[:], _pair2(wqh[:, off:off + 256]),
                     _pair2(wql[:, off:off + 256]), mh, ml,
                     p == 0, p == KT // 2 - 1)
                yield

        wo_state = {"obig": None, "pool": None, "bufs": 2}

        def wo_groups(c, atbs, lo_m, hi_m):
            # one group = one [128q, 512hid] output tile of chunk c
            for m in range(lo_m, hi_m):
                wj, wn = m // 8, m % 8
                if wn % 2 == 0:
                    wo_state["obig"] = posb.tile([128, 1024], BF16,
                                                 tag="osb", name="osb")
                obig = wo_state["obig"]
                ath_t, atl_t = atbs
                po = wo_state["pool"].tile([128, 512], F32, tag="po",
                                            name="po", bufs=wo_state["bufs"])
                for hp in range(2):
                    soff = 1024 * hp + 256 * wj
                    moff = 8192 * hp + 1024 * wn
                    sm = _pair2(ath_t[:, soff:soff + 256])
                    sl_ = _pair2(atl_t[:, soff:soff + 256])
                    mh = _pair2(woh[:, moff:moff + 1024])
                    ml = _pair2(wol[:, moff:moff + 1024])
                    _mm3(po[:], sm, sl_, mh, ml, hp == 0, hp == 1)
                nc.scalar.activation(obig[:, 512 * (wn % 2):512 * (wn % 2 + 1)],
                                     po[:], IDN, scale=WSC)
                if wn % 2 == 1:
                    nc.scalar.dma_start(
                        out[SC * c + 128 * wj:SC * c + 128 * (wj + 1),
                            1024 * (wn // 2):1024 * (wn // 2 + 1)],
                        obig[:])
                yield

        def _at_head_ap(t, h):
            hp, pl = h // 2, h % 2
            off = 1024 * hp + 128 * pl
            return t[:, off:off + 1024].rearrange(
                "p (b s) -> p b s", b=4)[:, :, 0:128]

        def attn_head(c, h, psc, ppv, atbs_out):
            kbs = _chunk_kbs(c)
            first_kb, last_kb = kbs[0], kbs[-1]
            acc_kbs = kbs
            tail_kbs = ()
            dacc = pdac.tile([128, 512], BF16, tag="dac", name="dac", bufs=2)
            nc.vector.memset(dacc[:], 0.0)
            den = pdnt[32 * (h % 2):32 * (h % 2) + 1, :]
            pvt = ppv.tile([128, 512], F32, tag="pv", name="pv", bufs=1)

            def emit_pv(kb, pt):
                lo, hi = _slot_region(kb - 4 * c + 8)
                nc.tensor.matmul(pvt[:, lo:hi], vnat[:, 128 * kb:128 * (kb + 1)],
                                 pt[:, lo:hi], start=(kb == first_kb),
                                 stop=(kb == last_kb), skip_group_check=True)
                if kb in tail_kbs:
                    nc.tensor.matmul(den[:, lo:hi], ones_t[:], pt[:, lo:hi],
                                     start=False, stop=(kb == last_kb),
                                     skip_group_check=True)

            pending = []
            for kb in kbs:
                sl = kb - 4 * c + 8
                lo, hi = _slot_region(sl)
                sct = psc.tile([128, 512], F32, tag="sc", name="sc", bufs=2)
                nc.tensor.matmul(sct[:, lo:hi], ktb[:, 128 * kb:128 * (kb + 1)],
                                 qtb[h][:, SC * c + lo:SC * c + hi],
                                 start=True, stop=True)
                pt = ppt.tile([128, 512], BF16, tag="pt", name="pt", bufs=6)
                nc.scalar.activation(pt[:, lo:hi], sct[:, lo:hi], EXP, scale=SCALE)
                if sl <= 3:
                    mofs = 128 * sl
                    nc.vector.tensor_tensor(pt[:, mofs:mofs + 128],
                                            pt[:, mofs:mofs + 128], mw_t[:], MULT)
                elif sl >= 8:
                    mofs = 128 * (sl - 8)
                    nc.vector.tensor_tensor(pt[:, mofs:mofs + 128],
                                            pt[:, mofs:mofs + 128], mc_t[:], MULT)
                if kb in acc_kbs:
                    nc.vector.tensor_tensor(dacc[:, lo:hi], dacc[:, lo:hi],
                                            pt[:, lo:hi], ADD)
                pending.append((kb, pt))
                if len(pending) > 2:
                    emit_pv(*pending.pop(0))
                yield
            for item in pending:
                emit_pv(*item)
                yield
            # one ones-matmul folds the whole accumulated exp sum into the
            # denominator row (the PE wait on the last DVE add is absorbed
            # by the woven filler work)
            nc.tensor.matmul(den, ones_t[:], dacc[:], start=True, stop=True,
                             skip_group_check=True)
            # drain: stage P@V out via ACT so the bank frees, normalize off
            # the critical path
            pvu = pat.tile([128, 512], BF16, tag="pvu", name="pvu", bufs=2)
            nc.scalar.activation(pvu[:], pvt[:], IDN, scale=SA)
            dre = pmisc.tile([1, 512], BF16, tag="denr", name="denr")
            with nc.allow_low_precision(reason="softmax denom to bf16"):
                nc.vector.reciprocal(dre[:], den)
            dbc = pmisc.tile([128, 512], BF16, tag="denb", name="denb")
            nc.gpsimd.partition_broadcast(dbc[:], dre[:])
            at = pat.tile([128, 512], BF16, tag="atb", name="atb", bufs=2)
            nc.vector.tensor_tensor(at[:], pvu[:], dbc[:], MULT)
            ath_t, atl_t = atbs_out
            atv = at[:].rearrange("p (b s) -> p b s", s=128)
            hap = _at_head_ap(ath_t, h)
            nc.scalar.activation(hap, atv, COPY)
            nc.vector.tensor_tensor(_at_head_ap(atl_t, h), atv, hap, SUB)

        def chain(*gens):
            for g in gens:
                yield from g

        def drive(*gens_weights):
            gens = list(gens_weights)
            while gens:
                keep = []
                for g, wgt in gens:
                    alive = True
                    for _ in range(wgt):
                        try:
                            next(g)
                        except StopIteration:
                            alive = False
                            break
                    if alive:
                        keep.append((g, wgt))
                gens = keep

        def empty_gen():
            return iter(())

        # ---- initial DMA stream (weights + chunk-0 hidden tiles) ----
        # weight dram arrays are exact SBUF images (host pre-laid-out)
        for g in range(4):
            qs = 4096 * g
            ks = 1024 * g
            nc.sync.dma_start(wkh[:, ks:ks + 1024], d_wkh[:, ks:ks + 1024])
            nc.sync.dma_start(wkl[:, ks:ks + 1024], d_wkl[:, ks:ks + 1024])
            nc.sync.dma_start(wvh[:, ks:ks + 1024], d_wvh[:, ks:ks + 1024])
            nc.sync.dma_start(wvl[:, ks:ks + 1024], d_wvl[:, ks:ks + 1024])
            issue_hb(0, g)
            nc.sync.dma_start(wqh[:, qs:qs + 4096], d_wqh[:, qs:qs + 4096])
            nc.sync.dma_start(wql[:, qs:qs + 4096], d_wql[:, qs:qs + 4096])
        nc.sync.dma_start(cs_t[:], ctab[:])
        nc.sync.dma_start(sn_t[:], stab[:])
        nc.sync.dma_start(mc_t[:], mcaus[:])
        nc.sync.dma_start(mw_t[:], mwin[:])
        for g in range(4, 8):
            issue_hb(0, g)
        for j in range(8):
            js = 2048 * j
            nc.sync.dma_start(woh[:, js:js + 2048], d_woh[:, js:js + 2048])
            nc.sync.dma_start(wol[:, js:js + 2048], d_wol[:, js:js + 2048])

        # ---- bootstrap: KV sweep of chunk 0 ----
        pkv_pools = {}
        pkv_pools[0] = tc.alloc_tile_pool(name="kvps0", bufs=1, space="PSUM")
        drive((kv_sweep(0, pkv_pools[0]), 1))

        atbs_prev = None     # previous chunk's normalized attention tiles
        prev_c = None
        seq = [(it, c) for it in range(niter) for c in range(NCH)]
        for idx, (it, c) in enumerate(seq):
            nxt = seq[idx + 1] if idx + 1 < len(seq) else None
            kpp, vpp = kvt.pop(c)

            # V^T -> SBUF, DMA-transpose to natural; rope K. Both read the KV
            # PSUM banks, which then free for this chunk's attention pools.
            nc.scalar.activation(vtb[:, SC * c:SC * (c + 1)], vpp[:], IDN,
                                 scale=DSC)
            for b2 in range(4):
                bo = 128 * (4 * c + b2)
                nc.scalar.dma_start_transpose(
                    vnat[:, bo:bo + 128],
                    vtb[:, SC * c + 128 * b2:SC * c + 128 * (b2 + 1)])
            _rope(ktb[:, SC * c:SC * (c + 1)], kpp[:], c)
            pkv_pools.pop(c).release()

            pwo = tc.alloc_tile_pool(name=f"wops{idx}", bufs=2, space="PSUM")
            wo_state["pool"], wo_state["bufs"] = pwo, 2
            wo_iter = (wo_groups(prev_c, atbs_prev, 0, 32) if atbs_prev is not None
                       else empty_gen())
            # fp8 hi/lo attention-output planes, head-pair interleaved at
            # 128-col granularity (padded so the strided write AP fits)
            atbs_new = (pat.tile([128, 2304], FP8, tag="ath", name="ath", bufs=2),
                        pat.tile([128, 2304], FP8, tag="atl", name="atl", bufs=2))
            psc = ppv = None
            a01 = None
            for h in range(NQH):
                if nxt is not None:
                    issue_hb(nxt[1], 2 * h)
                    issue_hb(nxt[1], 2 * h + 1)
                qpp = pq.tile([128, 512], F32, tag="qp", name="qp", bufs=2)
                if h == 2:
                    # Wo groups are done after the h0/h1 sweeps: swap their
                    # 2 banks for a deeper score ring + double P@V buffers
                    pwo.release()
                    psc = tc.alloc_tile_pool(name=f"scps{idx}", bufs=2, space="PSUM", side="right")
                    ppv = tc.alloc_tile_pool(name=f"pvps{idx}", bufs=1, space="PSUM", side="right")
                    a01 = chain(attn_head(c, 0, psc, ppv, atbs_new),
                                attn_head(c, 1, psc, ppv, atbs_new))
                fill = wo_iter if h < 2 else a01
                drive((q_sweep(c, h, qpp), 2), (fill, 1))
                _rope(qtb[h][:, SC * c:SC * (c + 1)], qpp[:], c)
            # F: finish heads 0/1, run heads 2/3, weave next chunk's KV sweep
            a23 = chain(attn_head(c, 2, psc, ppv, atbs_new),
                        attn_head(c, 3, psc, ppv, atbs_new))
            if nxt is not None:
                pkv_pools[nxt[1]] = tc.alloc_tile_pool(
                    name=f"kvps{idx + 1}", bufs=1, space="PSUM")
                kv_fill = kv_sweep(nxt[1], pkv_pools[nxt[1]])
            else:
                kv_fill = empty_gen()
            drive((a01, 1), (wo_iter, 1), (a23, 2), (kv_fill, 2))
            ppv.release()
            psc.release()

            atbs_prev = atbs_new
            prev_c = c

        # drain the last chunk's Wo groups on a wider PSUM ring
        pdr = tc.alloc_tile_pool(name="drainps", bufs=4, space="PSUM")
        wo_state["pool"], wo_state["bufs"] = pdr, 4
        drive((wo_groups(prev_c, atbs_prev, 0, 32), 1))
        pdr.release()


_NC_CACHE = {}


def _build(niter=1, fused=True):
    import os
    fused = os.environ.get("KERNEL_FUSED", "1" if fused else "0") == "1"
    key = (niter, fused)
    if key in _NC_CACHE:
        return _NC_CACHE[key]
    nc = bacc.Bacc("TRN2", target_bir_lowering=False, debug=False,
                   enable_asserts=True, num_devices=NCORES)
    dr = {}

    def din(name, shape, dt=F32):
        dr[name] = nc.dram_tensor(name, shape, dt, kind="ExternalInput").ap()

    din("hth", [HID, S], FP8)
    din("htl", [HID, S], FP8)
    din("wqh", [128, KT * 512], FP8)
    din("wql", [128, KT * 512], FP8)
    din("wkh", [128, KT * 128], FP8)
    din("wkl", [128, KT * 128], FP8)
    din("wvh", [128, KT * 128], FP8)
    din("wvl", [128, KT * 128], FP8)
    din("woh", [128, NQH * HID], FP8)
    din("wol", [128, NQH * HID], FP8)
    din("ctab", [128, S])
    din("stab", [128, S])
    din("mcaus", [128, 128], BF16)
    din("mwin", [128, 128], BF16)
    out = nc.dram_tensor("out", [S, HID], BF16, kind="ExternalOutput").ap()

    with tile.TileContext(nc) as tc:
        _program(tc, dr, out, niter, fused)
    nc.compile()
    _NC_CACHE[key] = nc
    return nc


def make_in_maps(inputs):
    hs = np.asarray(inputs["hidden_states"], dtype=np.float32)
    Wq = np.asarray(inputs["Wq"], dtype=np.float32)
    Wk = np.asarray(inputs["Wk"], dtype=np.float32)
    Wv = np.asarray(inputs["Wv"], dtype=np.float32)
    Wo = np.asarray(inputs["Wo"], dtype=np.float32)
    pos = np.asarray(inputs["position_ids"]).reshape(-1)

    assert hs.shape == (1, S, HID), hs.shape
    H = hs[0]
    HT = np.ascontiguousarray(H.T)

    E4 = ml_dtypes.float8_e4m3

    def hilo(x):
        xh = x.astype(E4)
        xl = (x - xh.astype(np.float32)).astype(E4)
        return xh, xl

    # hidden-state hi/lo fp8 planes, scaled by SX
    hth, htl = hilo(HT * np.float32(SX))

    # RoPE tables in [d%64, s] layout; the 1/(SX*SWT) projection descale is
    # folded in (cos duplicated, sin sign-baked)
    inv = (1.0 / (ROPE_BASE ** (np.arange(0, D, 2, dtype=np.float32) / D))).astype(np.float32)
    ang = pos.astype(np.float32)[None, :] * inv[:, None]          # [64, S]
    dsc = np.float32(1.0 / (SX * SWT))
    cos64 = (np.cos(ang) * dsc).astype(np.float32)
    sin64 = (np.sin(ang) * dsc).astype(np.float32)
    ctab = np.concatenate([cos64, cos64], axis=0)                 # [128, S]
    stab = np.concatenate([-sin64, sin64], axis=0)                # sign-baked

    kk = np.arange(128)[:, None]
    qq = np.arange(128)[None, :]
    mcaus = (qq >= kk).astype(ml_dtypes.bfloat16)   # causal diag block, [k,q]
    mwin = (qq < kk).astype(ml_dtypes.bfloat16)     # window-edge block, [k,q]

    def q_image(Wshard):
        # SBUF image [128, KT*512]: col(pair, head, plane, j) =
        # 1024*pair + 256*head + 128*plane + j; partition = hid%128
        Ws = (Wshard.T * np.float32(SWT)).astype(np.float32)      # [HID, 512]
        X = Ws.reshape(KT // 2, 2, 128, NQH, 128)                 # [pr, pl, p, h, j]
        return np.ascontiguousarray(
            X.transpose(2, 0, 3, 1, 4).reshape(128, KT * 512))

    def kv_image(Wshard):
        # [128, KT*128]: col(pair, plane, j) = 256*pair + 128*plane + j
        Ws = (Wshard.T * np.float32(SWT)).astype(np.float32)      # [HID, 128]
        X = Ws.reshape(KT // 2, 2, 128, 128)                      # [pr, pl, p, j]
        return np.ascontiguousarray(
            X.transpose(2, 0, 1, 3).reshape(128, KT * 128))

    def o_image(Wshard_cols):
        # moving image [128, NQH*HID]: col(hp, wn, plane, j) =
        # 8192*hp + 1024*wn + 512*plane + j; partition = d within head
        Ws = (Wshard_cols.T * np.float32(SWT)).astype(np.float32)  # [512, HID]
        X = Ws.reshape(2, 2, 128, 8, 512)                          # [hp, pl, p, wn, j]
        return np.ascontiguousarray(
            X.transpose(2, 0, 3, 1, 4).reshape(128, NQH * HID))

    in_maps = []
    for i in range(NCORES):
        wqh, wql = hilo(q_image(Wq[512 * i:512 * (i + 1), :]))
        wkh, wkl = hilo(kv_image(Wk[128 * i:128 * (i + 1), :]))
        wvh, wvl = hilo(kv_image(Wv[128 * i:128 * (i + 1), :]))
        woh, wol = hilo(o_image(Wo[:, 512 * i:512 * (i + 1)]))
        in_maps.append({
            "hth": hth, "htl": htl,
            "wqh": wqh, "wql": wql,
            "wkh": wkh, "wkl": wkl,
            "wvh": wvh, "wvl": wvl,
            "woh": woh, "wol": wol,
            "ctab": ctab,
            "stab": stab,
            "mcaus": mcaus,
            "mwin": mwin,
        })

    return in_maps


def kernel(**inputs):
    in_maps = make_in_maps(inputs)
    nc = _build()
    res = run_bass_kernel_spmd(nc, in_maps, core_ids=list(range(NCORES)))

    acc = np.zeros((S, HID), dtype=np.float32)
    for r in res.results:
        acc += r["out"].astype(np.float32)
    return acc.reshape(1, S, HID)
